# revision 34
# baseline (speedup 1.0000x reference)
"""BCH/RS systematic encoder kernel for Trainium2 (8 NeuronCores, data parallel).

Computes out = concat([msg, (msg @ Gp) mod 2], axis=-1) for
msg [16384, 1000] f32 of 0/1 bits and Gp [1000, 256] f32 of 0/1 bits.

Design (per core, 2048 rows, 8 superchunks of 2x128):
  - HWDGE (sync) loads of msg f32 into the ld/st row tile A[:, :, 0:1000]
    (1 MB per call), all dispatched upfront; FIFO ring drains them in order
  - per-chunk cast f32 -> fp8e4 (0/1 exact) split across ACT/DVE; fp8 pairs
    live in 16-bit granules
  - PE transpose-mode on the granules (raw 2-byte movement via identity,
    bit-exact): pt[q, gb, m] = f[m, c, gb*128+q]; one DVE copy per chunk
    PSUM -> SBUF.  No DMA transposes: Tile serializes xbar DMAs against all
    other DMA traffic (HW deadlock guard), which kills overlap.
  - 8 accumulating fp8 matmuls per chunk read the granule tile with
    byte-strided APs (even/odd k); the k-interleave is folded into the
    host-side Gp slab layout (permuting the contraction index is free).
  - psum f32 -> i32 on ACT, AND 1 + i32 -> f32 parity on DVE
  - SWDGE (gpsimd) stores of full f32 rows, one per superchunk (the last
    one split per chunk to trim the final drain tail)
HBM traffic/core = 8.19 MB read + 10.29 MB write + 0.25 MB Gp (the minimum);
measured sustained DMA ~410 GB/s -> ~46 us of traffic.
"""

import os
import sys

import numpy as np

if os.path.isdir("/opt/trn_rl_repo") and "/opt/trn_rl_repo" not in sys.path:
    sys.path.insert(0, "/opt/trn_rl_repo")

import concourse.bacc as bacc
import concourse.mybir as mybir
import concourse.tile as tile
from concourse.bass_utils import run_bass_kernel_spmd
from concourse.masks import make_identity

BATCH = 16384
MSG = 1000
NPAR = 256
NCORES = 8
ROWS = BATCH // NCORES  # 2048
P = 128
KPAD = 1024  # fp8 columns after pad
NGRAN = KPAD // 2  # 512 16-bit granules
GB = NGRAN // P  # 4 granule blocks per chunk
OUTW = MSG + NPAR  # 1256

# test.py pokes these for profiling
TRACE = False
LAST_RESULT = None

_CACHE = {}


def build_nc(rows=ROWS):
    """Emit the Bass/Tile IR for one core handling `rows` rows."""
    nc = bacc.Bacc("TRN2", target_bir_lowering=False, debug=False)
    msg = nc.dram_tensor("msg", [rows, MSG], mybir.dt.float32, kind="ExternalInput")
    gp = nc.dram_tensor("gp", [P, 2 * GB * NPAR], mybir.dt.uint8, kind="ExternalInput")
    out = nc.dram_tensor("out", [rows, OUTW], mybir.dt.float32, kind="ExternalOutput")

    SC = 2  # chunks per superchunk
    n_super = rows // (SC * P)
    msg3 = msg[:, :].rearrange("(s c p) k -> s c p k", c=SC, p=P)
    out3 = out[:, :].rearrange("(s c p) k -> s c p k", c=SC, p=P)

    with tile.TileContext(nc) as tc:
        with (
            tc.tile_pool(name="gpool", bufs=1) as gpool,
            tc.tile_pool(name="apool", bufs=min(n_super, 8)) as apool,
            tc.tile_pool(name="fpool", bufs=4) as fpool,
            tc.tile_pool(name="tpool", bufs=4) as tpool,
            tc.tile_pool(name="cpool", bufs=3) as cpool,
            tc.tile_pool(name="epool", bufs=3) as epool,
            tc.tile_pool(name="ppool", bufs=2, space="PSUM") as ppool,
            tc.tile_pool(name="tpsum", bufs=3, space="PSUM") as tpsum,
        ):
            # Gp slabs resident in SBUF: row q of slab s=(gb,off) holds
            # Gp_pad[2*(128*gb+q)+off, :] as fp8 bytes
            gsb = gpool.tile([P, 2 * GB * NPAR], mybir.dt.uint8)
            nc.sync.dma_start(out=gsb[:, :], in_=gp[:, :])
            gsb8 = gsb[:, :].bitcast(mybir.dt.float8e4)
            # identity for PE transpose-mode (raw granule movement)
            ident = gpool.tile([P, P], mybir.dt.bfloat16)
            make_identity(nc, ident[:, :])

            a_tiles = {}

            def emit_load(si):
                # full output rows in f32: cols 0:1000 msg, 1000:1256 parity
                a = apool.tile([P, SC, OUTW], mybir.dt.float32, tag="a")
                nc.sync.dma_start(
                    out=a[:, :, 0:MSG], in_=msg3[si, :, :, :].rearrange("c p k -> p c k")
                )
                a_tiles[si] = a

            def emit_compute(si):
                a = a_tiles[si]
                # fp8 cast target, allocated as bf16 so the PE transpose sees
                # 2-byte granules; each granule = (fp8 k=2j, fp8 k=2j+1)
                f = fpool.tile([P, SC, NGRAN], mybir.dt.bfloat16, tag="f")
                f8 = f[:, :, :].bitcast(mybir.dt.float8e4)  # [P, SC, KPAD]
                # zero the pad so pad-row garbage can't turn into NaN*0 in PSUM
                nc.vector.memset(f[:, :, MSG // 2 :], 0)
                # per-chunk casts split across ACT and DVE
                for c in range(SC):
                    eng = nc.scalar.copy if c % 2 == 0 else nc.vector.tensor_copy
                    eng(f8[:, c, 0:MSG], a[:, c, 0:MSG])

                g = tpool.tile([P, SC * GB, P], mybir.dt.bfloat16, tag="g")
                # strided fp8 views: m stride = 2 bytes, off = byte offset
                g8 = g[:, :, :].bitcast(mybir.dt.float8e4).rearrange(
                    "q b (m two) -> q b two m", two=2
                )
                acc = ppool.tile([P, SC * NPAR], mybir.dt.float32, tag="acc")
                for c in range(SC):
                    pt = tpsum.tile([P, GB, P], mybir.dt.bfloat16, tag="pt")
                    for gb in range(GB):
                        nc.tensor.transpose(
                            pt[:, gb, :],
                            f[:, c, gb * P : (gb + 1) * P],
                            ident[:, :],
                        )
                    nc.vector.tensor_copy(
                        g[:, c * GB : (c + 1) * GB, :], pt[:, :, :]
                    )
                    for j in range(2 * GB):
                        gb, off = j // 2, j % 2
                        nc.tensor.matmul(
                            acc[:, c * NPAR : (c + 1) * NPAR],
                            g8[:, c * GB + gb, off, :],
                            gsb8[:, j * NPAR : (j + 1) * NPAR],
                            start=(j == 0),
                            stop=(j == 2 * GB - 1),
                        )
                # eviction: psum f32 -> i32 on ACT (exact), then AND 1 +
                # i32 -> f32 parity on DVE
                c_i32 = cpool.tile([P, SC, NPAR], mybir.dt.int32, tag="c")
                nc.scalar.copy(
                    c_i32[:, :, :].rearrange("p c n -> p (c n)"), acc[:, :]
                )
                e = epool.tile([P, SC, NPAR], mybir.dt.int32, tag="e")
                nc.vector.tensor_scalar(
                    e[:, :, :], c_i32[:, :, :], 1, None,
                    mybir.AluOpType.bitwise_and,
                )
                nc.vector.tensor_copy(a[:, :, MSG:OUTW], e[:, :, :])

            def emit_store(si, split=False):
                # SWDGE plain f32 stores from the idle gpsimd engine; the
                # last superchunk is split per chunk to trim the drain tail
                a = a_tiles.pop(si)
                if split:
                    for c in range(SC):
                        nc.gpsimd.dma_start(
                            out=out3[si, c, :, :], in_=a[:, c, 0:OUTW]
                        )
                else:
                    nc.gpsimd.dma_start(
                        out=out3[si, :, :, :].rearrange("c p k -> p c k"),
                        in_=a[:, :, 0:OUTW],
                    )

            for it in range(n_super):
                emit_load(it)
            for it in range(n_super):
                emit_compute(it)
                emit_store(it, split=(it == n_super - 1))

    nc.compile()
    return nc


def prep_gp(Gp):
    """Pad Gp to 1024 rows, interleave-permute k, and emit fp8 byte slabs.

    Slab s = gb*2 + off (gb in 0..3, off in 0..1); row q of slab s holds
    Gp_pad[2*(128*gb + q) + off, :] as fp8e4 bytes (1.0 -> 0x38).
    """
    gp = np.asarray(Gp, dtype=np.float32)
    gp_pad = np.zeros((KPAD, NPAR), dtype=np.float32)
    gp_pad[:MSG] = gp
    b = np.where(gp_pad > 0.5, np.uint8(0x38), np.uint8(0)).astype(np.uint8)
    # b[k, n], k = 2*(128*gb + q) + off -> [gb, q, off, n] -> [q, (gb, off), n]
    slabs = b.reshape(GB, P, 2, NPAR).transpose(1, 0, 2, 3).reshape(P, 2 * GB * NPAR)
    return np.ascontiguousarray(slabs)


def kernel(message_bits, Gp):
    global LAST_RESULT
    msg = np.ascontiguousarray(np.asarray(message_bits, dtype=np.float32))
    assert msg.shape == (BATCH, MSG), msg.shape
    gsw = prep_gp(Gp)

    if "nc" not in _CACHE:
        _CACHE["nc"] = build_nc()
    nc = _CACHE["nc"]

    in_maps = [
        {"msg": msg[i * ROWS : (i + 1) * ROWS], "gp": gsw} for i in range(NCORES)
    ]
    res = run_bass_kernel_spmd(
        nc, in_maps, core_ids=list(range(NCORES)), trace=TRACE
    )
    LAST_RESULT = res
    return np.concatenate([r["out"] for r in res.results], axis=0)


# revision 35
# speedup vs baseline: 1.0038x; 1.0038x over previous
"""BCH/RS systematic encoder kernel for Trainium2 (8 NeuronCores, data parallel).

Computes out = concat([msg, (msg @ Gp) mod 2], axis=-1) for
msg [16384, 1000] f32 of 0/1 bits and Gp [1000, 256] f32 of 0/1 bits.

Design (per core, 2048 rows, 8 superchunks of 2x128):
  - HWDGE (sync) loads of msg f32 into the ld/st row tile A[:, :, 0:1000]
    (1 MB per call), all dispatched upfront; FIFO ring drains them in order
  - per-chunk cast f32 -> fp8e4 (0/1 exact) split across ACT/DVE; fp8 pairs
    live in 16-bit granules
  - PE transpose-mode on the granules (raw 2-byte movement via identity,
    bit-exact): pt[q, gb, m] = f[m, c, gb*128+q]; one DVE copy per chunk
    PSUM -> SBUF.  No DMA transposes: Tile serializes xbar DMAs against all
    other DMA traffic (HW deadlock guard), which kills overlap.
  - 8 accumulating fp8 matmuls per chunk read the granule tile with
    byte-strided APs (even/odd k); the k-interleave is folded into the
    host-side Gp slab layout (permuting the contraction index is free).
  - psum f32 -> i32 on ACT, AND 1 + i32 -> f32 parity on DVE
  - SWDGE (gpsimd) stores of full f32 rows, one per superchunk (the last
    one split per chunk to trim the final drain tail)
HBM traffic/core = 8.19 MB read + 10.29 MB write + 0.25 MB Gp (the minimum);
measured sustained DMA ~410 GB/s -> ~46 us of traffic.
"""

import os
import sys

import numpy as np

if os.path.isdir("/opt/trn_rl_repo") and "/opt/trn_rl_repo" not in sys.path:
    sys.path.insert(0, "/opt/trn_rl_repo")

import concourse.bacc as bacc
import concourse.mybir as mybir
import concourse.tile as tile
from concourse.bass_utils import run_bass_kernel_spmd
from concourse.masks import make_identity

BATCH = 16384
MSG = 1000
NPAR = 256
NCORES = 8
ROWS = BATCH // NCORES  # 2048
P = 128
KPAD = 1024  # fp8 columns after pad
NGRAN = KPAD // 2  # 512 16-bit granules
GB = NGRAN // P  # 4 granule blocks per chunk
OUTW = MSG + NPAR  # 1256

# test.py pokes these for profiling
TRACE = False
LAST_RESULT = None

_CACHE = {}


def build_nc(rows=ROWS):
    """Emit the Bass/Tile IR for one core handling `rows` rows."""
    nc = bacc.Bacc("TRN2", target_bir_lowering=False, debug=False)
    msg = nc.dram_tensor("msg", [rows, MSG], mybir.dt.float32, kind="ExternalInput")
    gp = nc.dram_tensor("gp", [P, 2 * GB * NPAR], mybir.dt.uint8, kind="ExternalInput")
    out = nc.dram_tensor("out", [rows, OUTW], mybir.dt.float32, kind="ExternalOutput")

    SC = 2  # chunks per superchunk
    n_super = rows // (SC * P)
    # row -> (partition, chunk) mapping is (s p c), NOT (s c p): partition p
    # then holds ADJACENT DRAM rows 2p, 2p+1, so each load descriptor is one
    # contiguous 8000 B run (and each store run 10048 B) instead of 4000 B —
    # half the descriptor count at the same bytes
    msg3 = msg[:, :].rearrange("(s p c) k -> s c p k", c=SC, p=P)
    out3 = out[:, :].rearrange("(s p c) k -> s c p k", c=SC, p=P)

    with tile.TileContext(nc) as tc:
        with (
            tc.tile_pool(name="gpool", bufs=1) as gpool,
            tc.tile_pool(name="apool", bufs=min(n_super, 8)) as apool,
            tc.tile_pool(name="fpool", bufs=4) as fpool,
            tc.tile_pool(name="tpool", bufs=4) as tpool,
            tc.tile_pool(name="cpool", bufs=3) as cpool,
            tc.tile_pool(name="epool", bufs=3) as epool,
            tc.tile_pool(name="ppool", bufs=2, space="PSUM") as ppool,
            tc.tile_pool(name="tpsum", bufs=3, space="PSUM") as tpsum,
        ):
            # Gp slabs resident in SBUF: row q of slab s=(gb,off) holds
            # Gp_pad[2*(128*gb+q)+off, :] as fp8 bytes
            gsb = gpool.tile([P, 2 * GB * NPAR], mybir.dt.uint8)
            nc.sync.dma_start(out=gsb[:, :], in_=gp[:, :])
            gsb8 = gsb[:, :].bitcast(mybir.dt.float8e4)
            # identity for PE transpose-mode (raw granule movement)
            ident = gpool.tile([P, P], mybir.dt.bfloat16)
            make_identity(nc, ident[:, :])

            a_tiles = {}

            def emit_load(si):
                # full output rows in f32: cols 0:1000 msg, 1000:1256 parity
                a = apool.tile([P, SC, OUTW], mybir.dt.float32, tag="a")
                nc.sync.dma_start(
                    out=a[:, :, 0:MSG], in_=msg3[si, :, :, :].rearrange("c p k -> p c k")
                )
                a_tiles[si] = a

            def emit_compute(si):
                a = a_tiles[si]
                # fp8 cast target, allocated as bf16 so the PE transpose sees
                # 2-byte granules; each granule = (fp8 k=2j, fp8 k=2j+1)
                f = fpool.tile([P, SC, NGRAN], mybir.dt.bfloat16, tag="f")
                f8 = f[:, :, :].bitcast(mybir.dt.float8e4)  # [P, SC, KPAD]
                # zero the pad so pad-row garbage can't turn into NaN*0 in PSUM
                nc.vector.memset(f[:, :, MSG // 2 :], 0)
                # per-chunk casts split across ACT and DVE
                for c in range(SC):
                    eng = nc.scalar.copy if c % 2 == 0 else nc.vector.tensor_copy
                    eng(f8[:, c, 0:MSG], a[:, c, 0:MSG])

                g = tpool.tile([P, SC * GB, P], mybir.dt.bfloat16, tag="g")
                # strided fp8 views: m stride = 2 bytes, off = byte offset
                g8 = g[:, :, :].bitcast(mybir.dt.float8e4).rearrange(
                    "q b (m two) -> q b two m", two=2
                )
                acc = ppool.tile([P, SC * NPAR], mybir.dt.float32, tag="acc")
                for c in range(SC):
                    pt = tpsum.tile([P, GB, P], mybir.dt.bfloat16, tag="pt")
                    for gb in range(GB):
                        nc.tensor.transpose(
                            pt[:, gb, :],
                            f[:, c, gb * P : (gb + 1) * P],
                            ident[:, :],
                        )
                    nc.vector.tensor_copy(
                        g[:, c * GB : (c + 1) * GB, :], pt[:, :, :]
                    )
                    for j in range(2 * GB):
                        gb, off = j // 2, j % 2
                        nc.tensor.matmul(
                            acc[:, c * NPAR : (c + 1) * NPAR],
                            g8[:, c * GB + gb, off, :],
                            gsb8[:, j * NPAR : (j + 1) * NPAR],
                            start=(j == 0),
                            stop=(j == 2 * GB - 1),
                        )
                # eviction: psum f32 -> i32 on ACT (exact), then AND 1 +
                # i32 -> f32 parity on DVE
                c_i32 = cpool.tile([P, SC, NPAR], mybir.dt.int32, tag="c")
                nc.scalar.copy(
                    c_i32[:, :, :].rearrange("p c n -> p (c n)"), acc[:, :]
                )
                e = epool.tile([P, SC, NPAR], mybir.dt.int32, tag="e")
                nc.vector.tensor_scalar(
                    e[:, :, :], c_i32[:, :, :], 1, None,
                    mybir.AluOpType.bitwise_and,
                )
                nc.vector.tensor_copy(a[:, :, MSG:OUTW], e[:, :, :])

            def emit_store(si, split=False):
                # SWDGE plain f32 stores from the idle gpsimd engine; the
                # last superchunk is split per chunk to trim the drain tail
                a = a_tiles.pop(si)
                if split:
                    for c in range(SC):
                        nc.gpsimd.dma_start(
                            out=out3[si, c, :, :], in_=a[:, c, 0:OUTW]
                        )
                else:
                    nc.gpsimd.dma_start(
                        out=out3[si, :, :, :].rearrange("c p k -> p c k"),
                        in_=a[:, :, 0:OUTW],
                    )

            for it in range(n_super):
                emit_load(it)
            for it in range(n_super):
                emit_compute(it)
                emit_store(it, split=(it == n_super - 1))

    nc.compile()
    return nc


def prep_gp(Gp):
    """Pad Gp to 1024 rows, interleave-permute k, and emit fp8 byte slabs.

    Slab s = gb*2 + off (gb in 0..3, off in 0..1); row q of slab s holds
    Gp_pad[2*(128*gb + q) + off, :] as fp8e4 bytes (1.0 -> 0x38).
    """
    gp = np.asarray(Gp, dtype=np.float32)
    gp_pad = np.zeros((KPAD, NPAR), dtype=np.float32)
    gp_pad[:MSG] = gp
    b = np.where(gp_pad > 0.5, np.uint8(0x38), np.uint8(0)).astype(np.uint8)
    # b[k, n], k = 2*(128*gb + q) + off -> [gb, q, off, n] -> [q, (gb, off), n]
    slabs = b.reshape(GB, P, 2, NPAR).transpose(1, 0, 2, 3).reshape(P, 2 * GB * NPAR)
    return np.ascontiguousarray(slabs)


def kernel(message_bits, Gp):
    global LAST_RESULT
    msg = np.ascontiguousarray(np.asarray(message_bits, dtype=np.float32))
    assert msg.shape == (BATCH, MSG), msg.shape
    gsw = prep_gp(Gp)

    if "nc" not in _CACHE:
        _CACHE["nc"] = build_nc()
    nc = _CACHE["nc"]

    in_maps = [
        {"msg": msg[i * ROWS : (i + 1) * ROWS], "gp": gsw} for i in range(NCORES)
    ]
    res = run_bass_kernel_spmd(
        nc, in_maps, core_ids=list(range(NCORES)), trace=TRACE
    )
    LAST_RESULT = res
    return np.concatenate([r["out"] for r in res.results], axis=0)


# revision 36
# speedup vs baseline: 1.0100x; 1.0062x over previous
"""BCH/RS systematic encoder kernel for Trainium2 (8 NeuronCores, data parallel).

Computes out = concat([msg, (msg @ Gp) mod 2], axis=-1) for
msg [16384, 1000] f32 of 0/1 bits and Gp [1000, 256] f32 of 0/1 bits.

Design (per core, 2048 rows, 8 superchunks of 2x128):
  - HWDGE (sync) loads of msg f32 into the ld/st row tile A[:, :, 0:1000]
    (1 MB per call), all dispatched upfront; FIFO ring drains them in order
  - per-chunk cast f32 -> fp8e4 (0/1 exact) split across ACT/DVE; fp8 pairs
    live in 16-bit granules
  - PE transpose-mode on the granules (raw 2-byte movement via identity,
    bit-exact): pt[q, gb, m] = f[m, c, gb*128+q]; one DVE copy per chunk
    PSUM -> SBUF.  No DMA transposes: Tile serializes xbar DMAs against all
    other DMA traffic (HW deadlock guard), which kills overlap.
  - 8 accumulating fp8 matmuls per chunk read the granule tile with
    byte-strided APs (even/odd k); the k-interleave is folded into the
    host-side Gp slab layout (permuting the contraction index is free).
  - psum f32 -> i32 on ACT, AND 1 + i32 -> f32 parity on DVE
  - SWDGE (gpsimd) stores of full f32 rows, one per superchunk (the last
    one split per chunk to trim the final drain tail)
HBM traffic/core = 8.19 MB read + 10.29 MB write + 0.25 MB Gp (the minimum);
measured sustained DMA ~410 GB/s -> ~46 us of traffic.
"""

import os
import sys

import numpy as np

if os.path.isdir("/opt/trn_rl_repo") and "/opt/trn_rl_repo" not in sys.path:
    sys.path.insert(0, "/opt/trn_rl_repo")

import concourse.bacc as bacc
import concourse.mybir as mybir
import concourse.tile as tile
from concourse.bass_utils import run_bass_kernel_spmd
from concourse.masks import make_identity

BATCH = 16384
MSG = 1000
NPAR = 256
NCORES = 8
ROWS = BATCH // NCORES  # 2048
P = 128
KPAD = 1024  # fp8 columns after pad
NGRAN = KPAD // 2  # 512 16-bit granules
GB = NGRAN // P  # 4 granule blocks per chunk
OUTW = MSG + NPAR  # 1256

# test.py pokes these for profiling
TRACE = False
LAST_RESULT = None

_CACHE = {}


def build_nc(rows=ROWS):
    """Emit the Bass/Tile IR for one core handling `rows` rows."""
    nc = bacc.Bacc("TRN2", target_bir_lowering=False, debug=False)
    msg = nc.dram_tensor("msg", [rows, MSG], mybir.dt.float32, kind="ExternalInput")
    gp = nc.dram_tensor("gp", [P, 2 * GB * NPAR], mybir.dt.uint8, kind="ExternalInput")
    out = nc.dram_tensor("out", [rows, OUTW], mybir.dt.float32, kind="ExternalOutput")

    SC = 2  # chunks per superchunk
    n_super = rows // (SC * P)
    # row -> (partition, chunk) mapping is (s p c), NOT (s c p): partition p
    # then holds ADJACENT DRAM rows 2p, 2p+1, so each load descriptor is one
    # contiguous 8000 B run (and each store run 10048 B) instead of 4000 B —
    # half the descriptor count at the same bytes
    msg3 = msg[:, :].rearrange("(s p c) k -> s c p k", c=SC, p=P)
    out3 = out[:, :].rearrange("(s p c) k -> s c p k", c=SC, p=P)

    with tile.TileContext(nc) as tc:
        with (
            tc.tile_pool(name="gpool", bufs=1) as gpool,
            tc.tile_pool(name="apool", bufs=min(n_super, 8)) as apool,
            tc.tile_pool(name="fpool", bufs=4) as fpool,
            tc.tile_pool(name="tpool", bufs=4) as tpool,
            tc.tile_pool(name="cpool", bufs=3) as cpool,
            tc.tile_pool(name="epool", bufs=3) as epool,
            tc.tile_pool(name="ppool", bufs=2, space="PSUM") as ppool,
            tc.tile_pool(name="tpsum", bufs=3, space="PSUM") as tpsum,
        ):
            # Gp slabs resident in SBUF: row q of slab s=(gb,off) holds
            # Gp_pad[2*(128*gb+q)+off, :] as fp8 bytes
            gsb = gpool.tile([P, 2 * GB * NPAR], mybir.dt.uint8)
            nc.sync.dma_start(out=gsb[:, :], in_=gp[:, :])
            gsb8 = gsb[:, :].bitcast(mybir.dt.float8e4)
            # identity for PE transpose-mode (raw granule movement)
            ident = gpool.tile([P, P], mybir.dt.bfloat16)
            make_identity(nc, ident[:, :])

            a_tiles = {}

            def emit_load(si):
                # full output rows in f32: cols 0:1000 msg, 1000:1256 parity
                a = apool.tile([P, SC, OUTW], mybir.dt.float32, tag="a")
                nc.sync.dma_start(
                    out=a[:, :, 0:MSG], in_=msg3[si, :, :, :].rearrange("c p k -> p c k")
                )
                a_tiles[si] = a

            def emit_compute(si):
                a = a_tiles[si]
                # fp8 cast target, allocated as bf16 so the PE transpose sees
                # 2-byte granules; each granule = (fp8 k=2j, fp8 k=2j+1)
                f = fpool.tile([P, SC, NGRAN], mybir.dt.bfloat16, tag="f")
                f8 = f[:, :, :].bitcast(mybir.dt.float8e4)  # [P, SC, KPAD]
                # zero the pad so pad-row garbage can't turn into NaN*0 in PSUM
                nc.vector.memset(f[:, :, MSG // 2 :], 0)
                # per-chunk casts split across ACT and DVE
                for c in range(SC):
                    eng = nc.scalar.copy if c % 2 == 0 else nc.vector.tensor_copy
                    eng(f8[:, c, 0:MSG], a[:, c, 0:MSG])

                g = tpool.tile([P, SC * GB, P], mybir.dt.bfloat16, tag="g")
                # strided fp8 views: m stride = 2 bytes, off = byte offset
                g8 = g[:, :, :].bitcast(mybir.dt.float8e4).rearrange(
                    "q b (m two) -> q b two m", two=2
                )
                acc = ppool.tile([P, SC * NPAR], mybir.dt.float32, tag="acc")
                for c in range(SC):
                    pt = tpsum.tile([P, GB, P], mybir.dt.bfloat16, tag="pt")
                    for gb in range(GB):
                        nc.tensor.transpose(
                            pt[:, gb, :],
                            f[:, c, gb * P : (gb + 1) * P],
                            ident[:, :],
                        )
                    nc.vector.tensor_copy(
                        g[:, c * GB : (c + 1) * GB, :], pt[:, :, :]
                    )
                    for j in range(2 * GB):
                        gb, off = j // 2, j % 2
                        nc.tensor.matmul(
                            acc[:, c * NPAR : (c + 1) * NPAR],
                            g8[:, c * GB + gb, off, :],
                            gsb8[:, j * NPAR : (j + 1) * NPAR],
                            start=(j == 0),
                            stop=(j == 2 * GB - 1),
                        )
                # eviction: psum f32 -> i32 on ACT (exact), then AND 1 +
                # i32 -> f32 parity on DVE
                c_i32 = cpool.tile([P, SC, NPAR], mybir.dt.int32, tag="c")
                nc.scalar.copy(
                    c_i32[:, :, :].rearrange("p c n -> p (c n)"), acc[:, :]
                )
                e = epool.tile([P, SC, NPAR], mybir.dt.int32, tag="e")
                nc.vector.tensor_scalar(
                    e[:, :, :], c_i32[:, :, :], 1, None,
                    mybir.AluOpType.bitwise_and,
                )
                nc.vector.tensor_copy(a[:, :, MSG:OUTW], e[:, :, :])

            def emit_store(si, split=False):
                # SWDGE plain f32 stores from the idle gpsimd engine; the
                # last superchunk is split per chunk to trim the drain tail
                a = a_tiles.pop(si)
                if split:
                    for c in range(SC):
                        nc.gpsimd.dma_start(
                            out=out3[si, c, :, :], in_=a[:, c, 0:OUTW]
                        )
                else:
                    nc.gpsimd.dma_start(
                        out=out3[si, :, :, :].rearrange("c p k -> p c k"),
                        in_=a[:, :, 0:OUTW],
                    )

            for it in range(n_super):
                emit_load(it)
            for it in range(n_super):
                emit_compute(it)
                emit_store(it)

    nc.compile()
    return nc


def prep_gp(Gp):
    """Pad Gp to 1024 rows, interleave-permute k, and emit fp8 byte slabs.

    Slab s = gb*2 + off (gb in 0..3, off in 0..1); row q of slab s holds
    Gp_pad[2*(128*gb + q) + off, :] as fp8e4 bytes (1.0 -> 0x38).
    """
    gp = np.asarray(Gp, dtype=np.float32)
    gp_pad = np.zeros((KPAD, NPAR), dtype=np.float32)
    gp_pad[:MSG] = gp
    b = np.where(gp_pad > 0.5, np.uint8(0x38), np.uint8(0)).astype(np.uint8)
    # b[k, n], k = 2*(128*gb + q) + off -> [gb, q, off, n] -> [q, (gb, off), n]
    slabs = b.reshape(GB, P, 2, NPAR).transpose(1, 0, 2, 3).reshape(P, 2 * GB * NPAR)
    return np.ascontiguousarray(slabs)


def kernel(message_bits, Gp):
    global LAST_RESULT
    msg = np.ascontiguousarray(np.asarray(message_bits, dtype=np.float32))
    assert msg.shape == (BATCH, MSG), msg.shape
    gsw = prep_gp(Gp)

    if "nc" not in _CACHE:
        _CACHE["nc"] = build_nc()
    nc = _CACHE["nc"]

    in_maps = [
        {"msg": msg[i * ROWS : (i + 1) * ROWS], "gp": gsw} for i in range(NCORES)
    ]
    res = run_bass_kernel_spmd(
        nc, in_maps, core_ids=list(range(NCORES)), trace=TRACE
    )
    LAST_RESULT = res
    return np.concatenate([r["out"] for r in res.results], axis=0)


# revision 38
# speedup vs baseline: 1.0322x; 1.0219x over previous
"""BCH/RS systematic encoder kernel for Trainium2 (8 NeuronCores, data parallel).

Computes out = concat([msg, (msg @ Gp) mod 2], axis=-1) for
msg [16384, 1000] f32 of 0/1 bits and Gp [1000, 256] f32 of 0/1 bits.

Design (per core, 2048 rows, 8 superchunks of 2x128):
  - HWDGE (sync) loads of msg f32 into the ld/st row tile A[:, :, 0:1000]
    (1 MB per call), all dispatched upfront; FIFO ring drains them in order
  - per-chunk cast f32 -> fp8e4 (0/1 exact) split across ACT/DVE; fp8 pairs
    live in 16-bit granules
  - PE transpose-mode on the granules (raw 2-byte movement via identity,
    bit-exact): pt[q, gb, m] = f[m, c, gb*128+q]; one DVE copy per chunk
    PSUM -> SBUF.  No DMA transposes: Tile serializes xbar DMAs against all
    other DMA traffic (HW deadlock guard), which kills overlap.
  - 8 accumulating fp8 matmuls per chunk read the granule tile with
    byte-strided APs (even/odd k); the k-interleave is folded into the
    host-side Gp slab layout (permuting the contraction index is free).
  - psum f32 -> i32 on ACT, AND 1 + i32 -> f32 parity on DVE
  - SWDGE (gpsimd) stores of full f32 rows, one per superchunk
HBM traffic/core = 8.19 MB read + 10.29 MB write + 0.25 MB Gp (the minimum);
measured sustained DMA ~410 GB/s -> ~46 us of traffic.
"""

import os
import sys

import numpy as np

if os.path.isdir("/opt/trn_rl_repo") and "/opt/trn_rl_repo" not in sys.path:
    sys.path.insert(0, "/opt/trn_rl_repo")

import concourse.bacc as bacc
import concourse.mybir as mybir
import concourse.tile as tile
from concourse.bass_utils import run_bass_kernel_spmd
from concourse.masks import make_identity

BATCH = 16384
MSG = 1000
NPAR = 256
NCORES = 8
ROWS = BATCH // NCORES  # 2048
P = 128
KPAD = 1024  # fp8 columns after pad
NGRAN = KPAD // 2  # 512 16-bit granules
GB = NGRAN // P  # 4 granule blocks per chunk
OUTW = MSG + NPAR  # 1256

# test.py pokes these for profiling
TRACE = False
LAST_RESULT = None

_CACHE = {}


def build_nc(rows=ROWS):
    """Emit the Bass/Tile IR for one core handling `rows` rows."""
    nc = bacc.Bacc("TRN2", target_bir_lowering=False, debug=False)
    msg = nc.dram_tensor("msg", [rows, MSG], mybir.dt.float32, kind="ExternalInput")
    gp = nc.dram_tensor("gp", [P, 2 * GB * NPAR], mybir.dt.uint8, kind="ExternalInput")
    out = nc.dram_tensor("out", [rows, OUTW], mybir.dt.float32, kind="ExternalOutput")

    SC = 4 if rows % (4 * P) == 0 else 2  # chunks per superchunk
    n_super = rows // (SC * P)
    # row -> (partition, chunk) mapping is (s p c), NOT (s c p): partition p
    # then holds ADJACENT DRAM rows 2p, 2p+1, so each load descriptor is one
    # contiguous 8000 B run (and each store run 10048 B) instead of 4000 B —
    # half the descriptor count at the same bytes
    msg3 = msg[:, :].rearrange("(s p c) k -> s c p k", c=SC, p=P)
    out3 = out[:, :].rearrange("(s p c) k -> s c p k", c=SC, p=P)

    with tile.TileContext(nc) as tc:
        with (
            tc.tile_pool(name="gpool", bufs=1) as gpool,
            tc.tile_pool(name="apool", bufs=min(n_super, 8)) as apool,
            tc.tile_pool(name="fpool", bufs=4) as fpool,
            tc.tile_pool(name="tpool", bufs=4) as tpool,
            tc.tile_pool(name="cpool", bufs=3) as cpool,
            tc.tile_pool(name="epool", bufs=3) as epool,
            tc.tile_pool(name="ppool", bufs=2, space="PSUM") as ppool,
            tc.tile_pool(name="tpsum", bufs=3, space="PSUM") as tpsum,
        ):
            # Gp slabs resident in SBUF: row q of slab s=(gb,off) holds
            # Gp_pad[2*(128*gb+q)+off, :] as fp8 bytes
            gsb = gpool.tile([P, 2 * GB * NPAR], mybir.dt.uint8)
            nc.sync.dma_start(out=gsb[:, :], in_=gp[:, :])
            gsb8 = gsb[:, :].bitcast(mybir.dt.float8e4)
            # identity for PE transpose-mode (raw granule movement)
            ident = gpool.tile([P, P], mybir.dt.bfloat16)
            make_identity(nc, ident[:, :])

            a_tiles = {}

            def emit_load(si):
                # full output rows in f32: cols 0:1000 msg, 1000:1256 parity
                a = apool.tile([P, SC, OUTW], mybir.dt.float32, tag="a")
                nc.sync.dma_start(
                    out=a[:, :, 0:MSG], in_=msg3[si, :, :, :].rearrange("c p k -> p c k")
                )
                a_tiles[si] = a

            def emit_compute(si):
                a = a_tiles[si]
                # fp8 cast target, allocated as bf16 so the PE transpose sees
                # 2-byte granules; each granule = (fp8 k=2j, fp8 k=2j+1)
                f = fpool.tile([P, SC, NGRAN], mybir.dt.bfloat16, tag="f")
                f8 = f[:, :, :].bitcast(mybir.dt.float8e4)  # [P, SC, KPAD]
                # zero the pad so pad-row garbage can't turn into NaN*0 in PSUM
                nc.vector.memset(f[:, :, MSG // 2 :], 0)
                # per-chunk casts split across ACT and DVE
                for c in range(SC):
                    eng = nc.scalar.copy if c % 2 == 0 else nc.vector.tensor_copy
                    eng(f8[:, c, 0:MSG], a[:, c, 0:MSG])

                g = tpool.tile([P, SC * GB, P], mybir.dt.bfloat16, tag="g")
                # strided fp8 views: m stride = 2 bytes, off = byte offset
                g8 = g[:, :, :].bitcast(mybir.dt.float8e4).rearrange(
                    "q b (m two) -> q b two m", two=2
                )
                acc = ppool.tile([P, SC * NPAR], mybir.dt.float32, tag="acc")
                for c in range(SC):
                    pt = tpsum.tile([P, GB, P], mybir.dt.bfloat16, tag="pt")
                    for gb in range(GB):
                        nc.tensor.transpose(
                            pt[:, gb, :],
                            f[:, c, gb * P : (gb + 1) * P],
                            ident[:, :],
                        )
                    nc.vector.tensor_copy(
                        g[:, c * GB : (c + 1) * GB, :], pt[:, :, :]
                    )
                    for j in range(2 * GB):
                        gb, off = j // 2, j % 2
                        nc.tensor.matmul(
                            acc[:, c * NPAR : (c + 1) * NPAR],
                            g8[:, c * GB + gb, off, :],
                            gsb8[:, j * NPAR : (j + 1) * NPAR],
                            start=(j == 0),
                            stop=(j == 2 * GB - 1),
                        )
                # eviction: psum f32 -> i32 on ACT (exact), then AND 1 +
                # i32 -> f32 parity on DVE
                c_i32 = cpool.tile([P, SC, NPAR], mybir.dt.int32, tag="c")
                nc.scalar.copy(
                    c_i32[:, :, :].rearrange("p c n -> p (c n)"), acc[:, :]
                )
                e = epool.tile([P, SC, NPAR], mybir.dt.int32, tag="e")
                nc.vector.tensor_scalar(
                    e[:, :, :], c_i32[:, :, :], 1, None,
                    mybir.AluOpType.bitwise_and,
                )
                nc.vector.tensor_copy(a[:, :, MSG:OUTW], e[:, :, :])

            def emit_store(si):
                # SWDGE plain f32 store from the idle gpsimd engine; with the
                # (s p c) row mapping every descriptor is one contiguous
                # 10048 B run (two adjacent output rows)
                a = a_tiles.pop(si)
                nc.gpsimd.dma_start(
                    out=out3[si, :, :, :].rearrange("c p k -> p c k"),
                    in_=a[:, :, 0:OUTW],
                )

            for it in range(n_super):
                emit_load(it)
            for it in range(n_super):
                emit_compute(it)
                emit_store(it)

    nc.compile()
    return nc


def prep_gp(Gp):
    """Pad Gp to 1024 rows, interleave-permute k, and emit fp8 byte slabs.

    Slab s = gb*2 + off (gb in 0..3, off in 0..1); row q of slab s holds
    Gp_pad[2*(128*gb + q) + off, :] as fp8e4 bytes (1.0 -> 0x38).
    """
    gp = np.asarray(Gp, dtype=np.float32)
    gp_pad = np.zeros((KPAD, NPAR), dtype=np.float32)
    gp_pad[:MSG] = gp
    b = np.where(gp_pad > 0.5, np.uint8(0x38), np.uint8(0)).astype(np.uint8)
    # b[k, n], k = 2*(128*gb + q) + off -> [gb, q, off, n] -> [q, (gb, off), n]
    slabs = b.reshape(GB, P, 2, NPAR).transpose(1, 0, 2, 3).reshape(P, 2 * GB * NPAR)
    return np.ascontiguousarray(slabs)


def kernel(message_bits, Gp):
    global LAST_RESULT
    msg = np.ascontiguousarray(np.asarray(message_bits, dtype=np.float32))
    assert msg.shape == (BATCH, MSG), msg.shape
    gsw = prep_gp(Gp)

    if "nc" not in _CACHE:
        _CACHE["nc"] = build_nc()
    nc = _CACHE["nc"]

    in_maps = [
        {"msg": msg[i * ROWS : (i + 1) * ROWS], "gp": gsw} for i in range(NCORES)
    ]
    res = run_bass_kernel_spmd(
        nc, in_maps, core_ids=list(range(NCORES)), trace=TRACE
    )
    LAST_RESULT = res
    return np.concatenate([r["out"] for r in res.results], axis=0)
